# revision 19
# baseline (speedup 1.0000x reference)
"""Trainium2 Bass kernel for the pairwise-distance masked log-sum loss.

Reference math (N=8192 points, E=49152 edges):
    dist[i,j] = |p_i - p_j|^2 + 1e-8
    mask      = (dist <= 0.25), edges (both directions) and diagonal zeroed
    loss      = sum(-log(dist) * mask)

Device strategy (8 NeuronCores, SPMD):
  * Points are partitioned hierarchically: 4 x-bands x 4 y-blocks, each
    512-point cell z-sorted and cut into 128-row tiles.  A tile's window =
    its own 128 columns + all forward columns reachable within the 0.5
    threshold (cells filtered by x/y extents, columns z-trimmed inside
    each candidate cell).  Every unordered off-diagonal pair inside the
    threshold appears exactly once; the host doubles the device sum.
    The 64 tiles are snake-dealt (desc window width) to 8 cores; slot
    widths pad to the per-slot max (32-col rounding) so every core runs
    an identical SPMD program (pad columns hold a far-away dummy point).
  * dist[i,j] = w_i . u_j with KCH=18 fp16 split-precision channels on the
    TensorE (hi/lo mantissa splits reconstruct fp32-grade |p_i - p_j|^2).
  * Input is packed [54, X] (3 stripes x 18 channel rows); per-slot chunk
    DMAs stream in processing order so the first matmul starts as soon as
    slot 0's columns land, not after the whole input.
  * The per-core column stream (all slot windows back to back, 7.5K cols)
    is chopped into ACT/DVE groups: 512 first (fast pipeline start) then
    1536 (PSUM bank pairs), cuts never splitting an own-tile block.  Per
    group: matmuls -> ScalarE ln PSUM->SBUF bf16 -> GpSimd affine_select
    overwrites each own-tile lower triangle + diagonal with +50 (pushes
    them out of the mask) -> one fused DVE scalar_tensor_tensor computes
    sum(y * (y <= ln .25)) into a per-group accumulator.
  * Host: loss = -2*S_dev + 2*sum(ln dist over unique non-self edge pairs
    inside the threshold).
"""

import os

import numpy as np

N = 8192
NCORES = 8
ROW_TILE = 128
TILES = N // ROW_TILE  # 64
SLOTS = TILES // NCORES  # 8 row-tiles per core
KCH = 18  # fp16 split-precision channels
BANK = 512  # PSUM bank width (f32 cols)
GROUP_COLS = int(os.environ.get("KERNEL_GROUP_COLS", "1536"))
FIRST_GROUP = int(os.environ.get("KERNEL_FIRST_GROUP", "512"))
EPS = 1e-8
THR2 = 0.25
XWIN = 0.5
LN_THR = float(np.log(0.25))
DELTA = 6e-6  # positivity cushion folded into the u-side |p|^2 split
BX, BY = 4, 4  # x-bands x y-blocks (cells of 512 pts, z-sorted)
PAD_TO = 32

ACC_SLOTS = 64

LAST_RESULT = {}


def _rn(v: np.ndarray, bits: int) -> np.ndarray:
    """Round f32/f64 values to `bits` explicit mantissa bits (RN)."""
    v64 = np.asarray(v, dtype=np.float64)
    m, e = np.frexp(v64)
    q = np.ldexp(np.round(np.ldexp(m, bits + 1)) / (1 << (bits + 1)), e)
    return q.astype(np.float32)


def _build_channels(pts: np.ndarray):
    """w [KCH, n] and u [KCH, n] fp16-grid channel vectors such that
    sum_k w[k,i]*u[k,j] ~= |p_i - p_j|^2 (u side carries +DELTA so every
    distance, incl. the split-residual diagonal, stays positive for Ln)."""
    bits = 10
    c = np.asarray(pts, dtype=np.float32)
    ch = _rn(c, bits)
    cl = _rn(c.astype(np.float64) - ch, bits)
    rep = ch.astype(np.float64) + cl  # represented points
    sq = (rep * rep).sum(axis=1)  # f64, exact-ish
    squ = sq + DELTA

    n = c.shape[0]
    w = np.empty((KCH, n), np.float32)
    u = np.empty((KCH, n), np.float32)
    for a in range(3):
        w[4 * a + 0] = -2.0 * ch[:, a]
        u[4 * a + 0] = ch[:, a]
        w[4 * a + 1] = -2.0 * ch[:, a]
        u[4 * a + 1] = cl[:, a]
        w[4 * a + 2] = -2.0 * cl[:, a]
        u[4 * a + 2] = ch[:, a]
        w[4 * a + 3] = -2.0 * cl[:, a]
        u[4 * a + 3] = cl[:, a]
    k = 12
    for val, side in ((sq, "w"), (squ, "u")):
        rem = val.copy()
        for _ in range(3):
            hi = _rn(rem, bits)
            if side == "w":
                w[k] = hi
                u[k] = 1.0
            else:
                w[k] = 1.0
                u[k] = hi
            rem = rem - hi
            k += 1
    assert k == KCH
    return w, u


def _host_prep(pred_pos: np.ndarray):
    """Hierarchical sort (x-bands, y-blocks, z within cell), per-tile
    geometric windows, snake balance; build per-core in_maps and meta."""
    p = np.asarray(pred_pos, dtype=np.float32)
    per_b = N // BX
    per_c = per_b // BY
    tpc = per_c // ROW_TILE
    CUSH = 1e-3

    xi = np.argsort(p[:, 0], kind="stable")
    psx = p[xi]
    parts = []
    cells = []  # (xlo,xhi,ylo,yhi, z sorted, global col offset)
    off = 0
    for bx in range(BX):
        seg = psx[bx * per_b : (bx + 1) * per_b]
        yi = np.argsort(seg[:, 1], kind="stable")
        seg = seg[yi]
        for by in range(BY):
            blk = seg[by * per_c : (by + 1) * per_c]
            zi = np.argsort(blk[:, 2], kind="stable")
            blk = blk[zi]
            parts.append(blk)
            cells.append(
                dict(
                    x=(float(blk[:, 0].min()), float(blk[:, 0].max())),
                    y=(float(blk[:, 1].min()), float(blk[:, 1].max())),
                    z=blk[:, 2].astype(np.float64),
                    off=off,
                )
            )
            off += per_c
    ps = np.concatenate(parts)

    w, u = _build_channels(ps)

    tile_ranges = []  # per tile: list of (lo, hi) global column ranges
    for ci, c in enumerate(cells):
        for ti in range(tpc):
            t0 = c["off"] + ti * ROW_TILE
            tile = ps[t0 : t0 + ROW_TILE]
            xb = float(tile[:, 0].max())
            ya, yb = float(tile[:, 1].min()), float(tile[:, 1].max())
            za, zb = float(tile[:, 2].min()), float(tile[:, 2].max())
            ranges = [(t0, t0 + ROW_TILE)]  # own tile first (tril masked)
            # own cell, forward in z
            hi = c["off"] + int(
                np.searchsorted(c["z"], zb + XWIN + CUSH, side="right")
            )
            if hi > t0 + ROW_TILE:
                ranges.append((t0 + ROW_TILE, hi))
            for cj in range(ci + 1, len(cells)):
                c2 = cells[cj]
                if c2["x"][0] - xb >= XWIN - CUSH:
                    continue
                if (
                    c2["y"][0] - yb >= XWIN - CUSH
                    or ya - c2["y"][1] >= XWIN - CUSH
                ):
                    continue
                lo = int(np.searchsorted(c2["z"], za - XWIN - CUSH))
                hi = int(
                    np.searchsorted(c2["z"], zb + XWIN + CUSH, side="right")
                )
                if hi > lo:
                    ranges.append((c2["off"] + lo, c2["off"] + hi))
            tile_ranges.append(ranges)

    widths = [sum(hi - lo for lo, hi in r) for r in tile_ranges]

    # snake-deal tiles (desc width) to cores; slot s width = max in band
    rank = sorted(range(TILES), key=lambda t: -widths[t])
    assign = [[None] * SLOTS for _ in range(NCORES)]
    for s in range(SLOTS):
        band = rank[s * NCORES : (s + 1) * NCORES]
        cores = range(NCORES) if s % 2 == 0 else range(NCORES - 1, -1, -1)
        for t, c in zip(band, cores):
            assign[c][s] = t
    slot_w = []
    for s in range(SLOTS):
        wmax = max(widths[assign[c][s]] for c in range(NCORES))
        slot_w.append(int(np.ceil(wmax / PAD_TO)) * PAD_TO)

    # process the largest slot first so the kernel tail (last group's
    # ACT/DVE chain) is as short as possible
    perm = sorted(range(SLOTS), key=lambda s: -slot_w[s])
    slot_w = [slot_w[s] for s in perm]
    assign = [[assign[c][perm[s]] for s in range(SLOTS)] for c in range(NCORES)]

    # dummy far-away point channels (outside any threshold window)
    _, ud = _build_channels(np.array([[100.0, 0.0, 0.0]], np.float32))

    # Striped [54, X] packed layout: stripe q (packed rows [18q, 18q+18),
    # matmul-time partitions [32q, 32q+18)) carries a few slots' data
    # [rowsW_s | win_s]...  Greedy assignment keeps stripe lengths balanced.
    stripe_of = {}
    col_of = {}
    stripe_len = [0, 0, 0]
    for s in range(SLOTS):  # slots already sorted by descending width
        q = min(range(3), key=lambda i: stripe_len[i])
        stripe_of[s] = q
        col_of[s] = (stripe_len[q], stripe_len[q] + ROW_TILE)
        stripe_len[q] += ROW_TILE + slot_w[s]
    X = max(stripe_len)

    in_maps = []
    ymask = np.tril(np.full((128, 128), 50.0, np.float32)).astype(np.float16)
    for c in range(NCORES):
        inp = np.zeros((3 * KCH, X), np.float16)
        for s in range(SLOTS):
            t = assign[c][s]
            q = stripe_of[s]
            rw_off, win_off = col_of[s]
            r0 = t * ROW_TILE
            inp[18 * q : 18 * q + KCH, rw_off : rw_off + ROW_TILE] = w[
                :, r0 : r0 + ROW_TILE
            ]
            o = win_off
            for lo, hi in tile_ranges[t]:
                inp[18 * q : 18 * q + KCH, o : o + hi - lo] = u[:, lo:hi]
                o += hi - lo
            if o < win_off + slot_w[s]:
                inp[18 * q : 18 * q + KCH, o : win_off + slot_w[s]] = ud
        in_maps.append({"inp": np.ascontiguousarray(inp), "ym": ymask})

    meta = {
        "slot_w": slot_w,
        "width": X,
        "stripe_of": stripe_of,
        "col_of": col_of,
    }
    return in_maps, meta


def _pack_groups(slot_w):
    """Chop the per-core column stream (slot windows back to back) into
    ACT/DVE groups: FIRST_GROUP then GROUP_COLS, cuts never splitting an
    own-tile 128-col block (needed whole for the affine_select tril)."""
    total = sum(slot_w)
    own_starts = []
    pos = 0
    for wl in slot_w:
        own_starts.append(pos)
        pos += wl
    groups = []
    pos = 0
    first = True
    while pos < total:
        cap = FIRST_GROUP if first else GROUP_COLS
        end = min(pos + cap, total)
        for S in own_starts:
            if pos < S < end < S + ROW_TILE:
                end = S
                break
        groups.append((pos, end - pos))
        pos = end
        first = False
    return groups, own_starts


def _edge_correction(pred_pos: np.ndarray, edges: np.ndarray) -> float:
    """sum of ln(dist) over unique unordered non-self edge pairs inside the
    threshold (each such pair appears exactly once in the device sum)."""
    p = np.asarray(pred_pos, dtype=np.float32)
    e = np.asarray(edges, dtype=np.int64)
    e = e[e[:, 0] != e[:, 1]]
    e = np.sort(e, axis=1)
    e = np.unique(e, axis=0)
    d = p[e[:, 0]] - p[e[:, 1]]
    dist = (d * d).sum(axis=1, dtype=np.float32) + np.float32(EPS)
    m = dist <= np.float32(THR2)
    return float(np.log(dist[m].astype(np.float64)).sum())


def _build_program(meta):
    import concourse.bass as bass
    import concourse.tile as tile
    from concourse import mybir
    from contextlib import ExitStack

    f32 = mybir.dt.float32
    bf16 = mybir.dt.bfloat16
    f16 = mybir.dt.float16

    slot_w = meta["slot_w"]
    width = meta["width"]
    stripe_of = meta["stripe_of"]
    col_of = meta["col_of"]

    groups, own_starts = _pack_groups(slot_w)
    n_groups = len(groups)
    assert n_groups <= ACC_SLOTS
    meta["groups"] = groups

    def slot_at(pos):
        for s in range(SLOTS - 1, -1, -1):
            if own_starts[s] <= pos:
                return s
        raise AssertionError

    nc = bass.Bass("TRN2", target_bir_lowering=False, debug=False, num_devices=NCORES)
    inp_d = nc.dram_tensor("inp", [3 * KCH, width], f16, kind="ExternalInput").ap()
    ym_d = nc.dram_tensor("ym", [128, 128], f16, kind="ExternalInput").ap()
    acc_d = nc.dram_tensor("acc", [128, ACC_SLOTS], f32, kind="ExternalOutput").ap()

    with tile.TileContext(nc) as tc, ExitStack() as ctx:
        singles = ctx.enter_context(tc.tile_pool(name="singles", bufs=1))
        psums = ctx.enter_context(tc.tile_pool(name="psums", bufs=2, space="PSUM"))
        prime_ps = ctx.enter_context(
            tc.tile_pool(name="prime_ps", bufs=1, space="PSUM")
        )
        ys = ctx.enter_context(tc.tile_pool(name="ys", bufs=SLOTS))
        scraps = ctx.enter_context(tc.tile_pool(name="scraps", bufs=2))

        inp_s = singles.tile([128, width], f16)
        ym_s = singles.tile([128, 128], f16)
        acc_s = singles.tile([128, ACC_SLOTS], f32)
        prime_v = singles.tile([128, 1], f16)
        pp = prime_ps.tile([1, 1], f32)

        # processing order: smallest slot first (fast pipeline start: its
        # matmul+ln+add chain gates the first STT), then descending,
        # second-smallest last (short kernel tail).  Slots are sorted by
        # descending width, so:
        proc = [SLOTS - 1] + list(range(SLOTS - 2)) + [SLOTS - 2]

        # Per-slot chunk DMAs in processing order: a slot's matmuls only
        # wait for its own chunk, so compute starts as soon as the first
        # chunk lands.  All go down one sync-engine queue -> complete in
        # order.
        for i, s in enumerate(proc):
            q = stripe_of[s]
            c0, w0 = col_of[s]
            c1 = w0 + slot_w[s]
            nc.sync.dma_start(
                out=inp_s[32 * q : 32 * q + KCH, c0:c1],
                in_=inp_d[18 * q : 18 * q + KCH, c0:c1],
            )
            if i == 0:
                nc.sync.dma_start(out=ym_s, in_=ym_d)

        # prime DVE's view of the ym DMA so the per-slot tril adds only
        # carry their ACT wait
        nc.vector.tensor_copy(out=prime_v, in_=ym_s[:, 0:1])

        # per-slot groups (HW-proven sync structure): prime -> matmuls ->
        # ACT ln -> tril add -> fused STT reduce
        assert max(slot_w) <= GROUP_COLS
        for gi, s in enumerate(proc):
            q = stripe_of[s]
            p0 = 32 * q
            rw_off, win_off = col_of[s]
            cols = slot_w[s]
            lhsT = inp_s[p0 : p0 + KCH, rw_off : rw_off + ROW_TILE]
            # prime PE's view of this slot's chunk DMA: later matmuls only
            # carry their PSUM-reuse (ACT) wait
            nc.tensor.matmul(
                out=pp,
                lhsT=inp_s[p0 : p0 + KCH, rw_off : rw_off + 1],
                rhs=inp_s[p0 : p0 + KCH, rw_off : rw_off + 1],
                start=True,
                stop=True,
            )
            psum_t = psums.tile([128, GROUP_COLS], f32, tag="ps")
            k = 0
            while k < cols:
                k_end = min(k + BANK, cols)
                nc.tensor.matmul(
                    out=psum_t[:, k:k_end],
                    lhsT=lhsT,
                    rhs=inp_s[p0 : p0 + KCH, win_off + k : win_off + k_end],
                    start=True,
                    stop=True,
                )
                k = k_end
            y_t = ys.tile([128, GROUP_COLS], bf16, tag="y")
            nc.scalar.activation(
                out=y_t[:, :cols],
                in_=psum_t[:, :cols],
                func=mybir.ActivationFunctionType.Ln,
            )
            # +50 on the own-tile lower triangle and diagonal pushes those
            # y values out of the ln .25 mask
            nc.vector.tensor_tensor(
                out=y_t[:, :ROW_TILE],
                in0=y_t[:, :ROW_TILE],
                in1=ym_s,
                op=mybir.AluOpType.add,
            )
            scrap_t = scraps.tile([128, GROUP_COLS], bf16, tag="scrap")
            nc.vector.scalar_tensor_tensor(
                out=scrap_t[:, :cols],
                in0=y_t[:, :cols],
                scalar=LN_THR,
                in1=y_t[:, :cols],
                op0=mybir.AluOpType.is_le,
                op1=mybir.AluOpType.mult,
                accum_out=acc_s[:, gi : gi + 1],
            )
        meta["n_groups_used"] = SLOTS

        nc.sync.dma_start(out=acc_d[:, :SLOTS], in_=acc_s[:, :SLOTS])

    if os.environ.get("KERNEL_STRIP", "1") == "1":
        _strip_self_waits(nc, mybir)
    return nc


_SELF_WAIT_OPCODES = {
    "InstMatmult",
    "InstTensorScalarPtr",
    "InstActivation",
    "InstTensorTensor",
    "InstTensorReduce",
    "InstTensorCopy",
    "InstMemset",
    "InstTensorScalarAffineSelect",
}
_ENGINE_SEM_PREFIX = {
    "PE": "PE_",
    "ACT": "Activation_",
    "DVE": "DVE_",
    "POOL": "Pool_",
    "SP": "SP_",
}


def _strip_self_waits(nc, mybir):
    """Walrus caps sync-wait commands per instruction (1 for PE/DVE compute
    structs).  Make every instruction single-wait:
      * compute ops: drop same-engine self-waits (in-order engines make
        them vacuous);
      * DMACopy: drop cross-queue DMA-ordering waits (all SBUF regions
        involved here are disjoint);
      * compute ops still multi-wait (e.g. the STT waiting on both ACT and
        the GpSimd tril select): hoist extra waits into single-wait
        same-engine Drains inserted just before;
      * Drain (kernel tail): split into a chain of single-wait drains;
      * anything else left with >1 wait: fail loudly (do NOT guess).
    """
    for fn in nc.m.functions:
        for bb in fn.blocks:
            for inst in bb.instructions:
                si = inst.sync_info
                if si is None or not si.on_wait or len(si.on_wait) < 2:
                    continue
                tname = type(inst).__name__
                waits = list(si.on_wait)
                if tname == "InstDMACopy":
                    keep = [
                        w
                        for w in waits
                        if not w.ant_name.startswith(("DMAHW", "DMASW"))
                    ]
                elif tname in _SELF_WAIT_OPCODES:
                    eng = getattr(inst.engine, "name", str(inst.engine))
                    prefix = None
                    for k, v in _ENGINE_SEM_PREFIX.items():
                        if k in str(eng).upper():
                            prefix = v
                            break
                    if prefix is None:
                        continue
                    keep = [w for w in waits if not w.ant_name.startswith(prefix)]
                else:
                    continue
                if keep and len(keep) < len(waits):
                    inst.sync_info = mybir.SyncInfo(
                        on_wait=keep, on_update=si.on_update
                    )

    # split multi-wait tail drains into chains of single-wait drains
    split_id = 0
    for fn in nc.m.functions:
        for bb in fn.blocks:
            idx = 0
            insts = bb.instructions
            while idx < len(insts):
                inst = insts[idx]
                si = inst.sync_info
                tname = type(inst).__name__
                if (
                    tname == "InstDrain"
                    and si is not None
                    and si.on_wait
                    and len(si.on_wait) > 1
                ):
                    waits = list(si.on_wait)
                    inst.sync_info = mybir.SyncInfo(
                        on_wait=[waits[-1]], on_update=si.on_update
                    )
                    for w in waits[:-1]:
                        nd = mybir.InstDrain(
                            name=f"I-drainsplit-{split_id}",
                            ins=[],
                            outs=[],
                            bass_is_fusable=False,
                        )
                        split_id += 1
                        nd.engine = inst.engine
                        nd.sync_info = mybir.SyncInfo(on_wait=[w], on_update=[])
                        insts.insert(idx, nd)
                        idx += 1
                idx += 1

    for fn in nc.m.functions:
        for bb in fn.blocks:
            for inst in bb.instructions:
                si = inst.sync_info
                if si is not None and si.on_wait and len(si.on_wait) > 1:
                    if type(inst).__name__ in ("InstEventSemaphore",):
                        continue
                    raise RuntimeError(
                        f"{inst.name} ({type(inst).__name__}) still has "
                        f"{len(si.on_wait)} waits: "
                        f"{[w.ant_name for w in si.on_wait]}"
                    )


def _finalize(results, pred_pos, edges, n_groups) -> np.float32:
    # every unordered off-diagonal pair inside the threshold appears exactly
    # once in the device sum -> double it; edge pairs likewise.
    s_all = 0.0
    for r in results:
        s_all += r["acc"][:, :n_groups].astype(np.float64).sum()
    corr = _edge_correction(pred_pos, edges)
    return np.float32(-2.0 * s_all + 2.0 * corr)


def kernel(pred_pos: np.ndarray, edges: np.ndarray) -> np.ndarray:
    from concourse.bass_utils import run_bass_kernel_spmd

    in_maps, meta = _host_prep(pred_pos)
    nc = _build_program(meta)
    trace = os.environ.get("KERNEL_TRACE", "0") == "1"
    trace_cores = None
    if os.environ.get("KERNEL_TRACE_ALL", "0") == "1":
        trace_cores = list(range(NCORES))
    res = run_bass_kernel_spmd(
        nc,
        in_maps,
        core_ids=list(range(NCORES)),
        trace=trace,
        trace_cores=trace_cores,
    )
    LAST_RESULT["exec_time_ns"] = res.exec_time_ns
    LAST_RESULT["trace"] = res.instructions_and_trace
    LAST_RESULT["meta"] = meta

    return _finalize(res.results, pred_pos, edges, meta["n_groups_used"])
